# revision 37
# baseline (speedup 1.0000x reference)
"""Trainium2 Bass kernel for EquiMAB3-style attention block.

Reference computation (per batch b; B=8, N=M=512, S=4, D=L=256, H=4, dh=64):
  Qp = Q @ Wq.T + bq          [N,S,L]
  Kp = K @ Wk.T + bk          [M,S,L]
  Vp = K @ Wv.T + bv          [M,S,L]
  E[h,n,m] = sum_{s,j} Qp[n,s,h*64+j] Kp[m,s,h*64+j] / 16
  A = softmax_m(E)
  O[n,s,l=h*64+j] = Qp[n,s,l] + sum_m A[h,n,m] Vp[m,s,l]
  O = LN0(O)*g0+b0 ; O = O + relu(O @ Wo.T + bo) ; out = LN1(O)*g1+b1

Sharding: data-parallel over B across the 8 cores (one batch element each).

Device dataflow (per core), bf16 data, f32 psum:
  host passes qt/kt as [D, (s n)] feature-major with s-MAJOR token order so
  every matmul rhs is a contiguous SBUF slice (strided rhs runs ~2x slower),
  and wall pre-swizzled to the SBUF layout (contiguous 4KB/partition DMA).
  kt lands first (sync HWDGE ring, 2 halves); cst/wall/qt on the scalar ring.
  Warm matmuls (3-deep psum ring) keep the PE DVFS p-state at 2.4GHz through
  the input-DMA wait and the LN0 latency bubble (idle PE drops to 1.2GHz for
  the next 3us of matmuls).
  QpT/KpT [l, (s n)] via proj matmuls; VA [m, (h s j+1)] m-major Vp with
  embedded ones-columns (softmax denominators ride the AV matmul for free)
  QN = PE transposes of QpT evacuated into o_sb early (plain copy)
  Et[h] [m, n] K=64 matmuls accumulated over s (head pairs on disjoint
  partition halves emit interleaved); exp on ACT
  AVn [n, (s j)]; evac = scalar_tensor_tensor (pa*recip + o_sb) fused add
  LN0 per nch: bn_stats; sqrt+recip; ybf=(o*rstd+nmr) on ACT (V holds stats)
  O1T = PE-transpose(ybf) with g0/b0 affine evac
  fc: psum = WoT-lhsT x O1T-rhs; relu evac to rt tiles (no residual add!)
  O2 = o1t^T + rt^T summed IN PSUM via paired X^T@I matmuls (f32, exact);
  LN1 stats on psum (V); apply on ACT -> bf16 out, one DMA per nch (sync)
"""

import os
import numpy as np

B, N, M, S, D, L, H = 8, 512, 512, 4, 256, 256, 4
DH = L // H  # 64
T = N * S  # 2048 tokens per core
P = 128
NCORES = 8

_CACHE = {}


def _build(use_mask, bias_zero, aff1_trivial):
    from contextlib import ExitStack

    import concourse.bacc as bacc
    import concourse.bass as bass
    import concourse.mybir as mybir
    import concourse.tile as tile
    from concourse.masks import make_identity

    f32 = mybir.dt.float32
    b16 = mybir.dt.bfloat16
    f8 = mybir.dt.float8e4
    AF = mybir.ActivationFunctionType
    OP = mybir.AluOpType

    nc = bacc.Bacc(
        "TRN2",
        target_bir_lowering=False,
        debug=False,
        enable_asserts=False,
        num_devices=NCORES,
    )

    def dram(name, shape, kind="ExternalInput", dt=None):
        return nc.dram_tensor(name, shape, dt or f32, kind=kind).ap()

    qt_d = dram("qt", [D, T], dt=b16)
    kt_d = dram("kt", [D, T], dt=b16)
    wall_d = dram("wall", [P, 2 * 4 * L], dt=b16)  # [p, (c w l)] contiguous
    cst_d = dram("cst", [P, 10])  # cols: bq(2) bk(2) bo(2) g0(2) b0(2)
    if not bias_zero:
        bv_d = dram("bvr", [L], dt=b16)
    if not aff1_trivial:
        g1_d = dram("g1v", [L])
        b1_d = dram("b1v", [L])
    if use_mask:
        mbt_d = dram("mbt", [M, N])  # (mask==0 ? -1e30 : 0), transposed [m, n]
    out_d = dram("o", [N, S * L], kind="ExternalOutput", dt=b16)

    def mm(out, lhsT, rhs, **kw):
        nc.tensor.matmul(out, lhsT, rhs, **kw)

    with tile.TileContext(nc) as tc, ExitStack() as ctx:
        singles = ctx.enter_context(tc.tile_pool(name="singles", bufs=1))
        io = ctx.enter_context(tc.tile_pool(name="io", bufs=2))
        qpt_pool = ctx.enter_context(tc.tile_pool(name="qpt", bufs=4))
        va_pool = ctx.enter_context(tc.tile_pool(name="va", bufs=4))
        at_pool = ctx.enter_context(tc.tile_pool(name="at", bufs=16))
        opool = ctx.enter_context(tc.tile_pool(name="opool", bufs=4))
        ybfp = ctx.enter_context(tc.tile_pool(name="ybfp", bufs=4))
        outp = ctx.enter_context(tc.tile_pool(name="outp", bufs=4))
        fmaj = ctx.enter_context(tc.tile_pool(name="fmaj", bufs=4))
        stat = ctx.enter_context(tc.tile_pool(name="stat", bufs=1))
        ppw = ctx.enter_context(tc.tile_pool(name="ppw", bufs=4, space="PSUM"))
        ppb = ctx.enter_context(tc.tile_pool(name="ppb", bufs=2, space="PSUM"))
        pp256 = ctx.enter_context(tc.tile_pool(name="pp256", bufs=2, space="PSUM"))

        # ---- input DMAs first: kt gates the first compute, then qt ----
        kt_sb = io.tile([P, 2, T], b16, name="kt")
        qt_sb = io.tile([P, 2, T], b16, name="qt")
        kt_r = kt_d.rearrange("(c p) t -> p c t", p=P)
        qt_r = qt_d.rearrange("(c p) t -> p c t", p=P)
        for hh in range(2):
            nc.sync.dma_start(out=kt_sb[:, :, hh * 1024:(hh + 1) * 1024],
                              in_=kt_r[:, :, hh * 1024:(hh + 1) * 1024])
        kt_v = [kt_sb[:, c, :] for c in range(2)]
        qt_v = [qt_sb[:, c, :] for c in range(2)]

        cst_sb = singles.tile([P, 10], f32)
        nc.scalar.dma_start(out=cst_sb, in_=cst_d)
        bq_c = cst_sb[:, 0:2]
        bk_c = cst_sb[:, 2:4]
        bo_c = cst_sb[:, 4:6]
        g0_c = cst_sb[:, 6:8]
        b0_c = cst_sb[:, 8:10]

        wall_sb = singles.tile([P, 2, 4, L], b16)
        nc.scalar.dma_start(out=wall_sb,
                            in_=wall_d.rearrange("p (c w l) -> p c w l",
                                                 c=2, w=4))
        for hh in range(2):
            nc.scalar.dma_start(out=qt_sb[:, :, hh * 1024:(hh + 1) * 1024],
                              in_=qt_r[:, :, hh * 1024:(hh + 1) * 1024])
        wq_sb = wall_sb[:, :, 0, :]
        wk_sb = wall_sb[:, :, 1, :]
        wv_sb = wall_sb[:, :, 2, :]
        wo_sb = wall_sb[:, :, 3, :]

        if not bias_zero:
            bv_bc = singles.tile([P, 2, L], b16)
            bv_rep = bass.AP(tensor=bv_d.tensor, offset=bv_d.offset,
                             ap=[[0, P]] + [list(dd) for dd in bv_d.ap])
            nc.gpsimd.dma_start(out=bv_bc[:, 0, :], in_=bv_rep)
            nc.gpsimd.dma_start(out=bv_bc[:, 1, :], in_=bv_rep)
        if not aff1_trivial:
            g1_bc = singles.tile([P, L], f32)
            b1_bc = singles.tile([P, L], f32)
            g1_rep = bass.AP(tensor=g1_d.tensor, offset=g1_d.offset,
                             ap=[[0, P]] + [list(d) for d in g1_d.ap])
            b1_rep = bass.AP(tensor=b1_d.tensor, offset=b1_d.offset,
                             ap=[[0, P]] + [list(d) for d in b1_d.ap])
            nc.gpsimd.dma_start(out=g1_bc, in_=g1_rep)
            nc.gpsimd.dma_start(out=b1_bc, in_=b1_rep)
        if use_mask:
            mb_sb = [singles.tile([P, N], f32, name=f"mb{i}") for i in range(4)]
            for mc in range(4):
                nc.gpsimd.dma_start(out=mb_sb[mc], in_=mbt_d[mc * P:(mc + 1) * P, :])

        ident_r = singles.tile([P, P], b16)
        make_identity(nc, ident_r)
        zero_col = singles.tile([P, 1], f32)
        nc.vector.memset(zero_col, 0.0)
        eps_col = singles.tile([P, 1], f32)
        nc.vector.memset(eps_col, 1e-5)
        warm_sb = singles.tile([P, 512], b16)
        nc.vector.memset(warm_sb, 0.0)

        def warm(n):
            for _ in range(n):
                pw = ppw.tile([P, 512], f32, tag="pw", name="pw")
                nc.tensor.matmul(pw, warm_sb[:, 0:P], warm_sb,
                                 start=True, stop=True)

        warm(30)

        # ---- projections: [l, (s n)] = W^T.T @ X^T + bias cols ----
        qpt_sb = [qpt_pool.tile([P, T], b16, name=f"qpt{i}") for i in range(2)]
        qpt8_sb = [qpt_pool.tile([P, 2, 2, 512], f8, name=f"qpt8{i}")
                   for i in range(2)]
        kpt8_sb = [qpt_pool.tile([P, 2, 2, 512], f8, name=f"kpt8{i}")
                   for i in range(2)]

        def proj(xt_v, w, bias_c, dst, dst8, veng, srange):
            for lc in range(2):
                for s in srange:
                    pg = ppw.tile([P, 512], f32, tag="pw")
                    for dc in range(2):
                        mm(pg, w[:, dc, lc * P:(lc + 1) * P],
                           xt_v[dc][:, s * 512:(s + 1) * 512],
                           start=(dc == 0), stop=(dc == 1))
                    bias_col = bias_c[:, lc:lc + 1]
                    d8 = dst8[lc][:, s // 2, s % 2, :]
                    if dst is not None:
                        dcols = dst[lc][:, s * 512:(s + 1) * 512]
                        if veng:
                            nc.vector.tensor_scalar(out=dcols, in0=pg,
                                                    scalar1=bias_col,
                                                    scalar2=None, op0=OP.add)
                            nc.scalar.activation(d8, pg, AF.Identity,
                                                 bias=bias_col)
                        else:
                            nc.scalar.activation(dcols, pg, AF.Identity,
                                                 bias=bias_col)
                            nc.vector.tensor_scalar(out=d8, in0=pg,
                                                    scalar1=bias_col,
                                                    scalar2=None, op0=OP.add)
                    else:
                        if veng:
                            nc.vector.tensor_scalar(out=d8, in0=pg,
                                                    scalar1=bias_col,
                                                    scalar2=None, op0=OP.add)
                        else:
                            nc.scalar.activation(d8, pg, AF.Identity,
                                                 bias=bias_col)
                    veng = not veng

        proj(kt_v, wk_sb, bk_c, None, kpt8_sb, True, (0, 1))

        # ---- VA (m-major Vp, no bias) layout [m, (h s j+1)] ----
        va_sb = [va_pool.tile([P, H, S, DH + 1], b16, name=f"va{i}")
                 for i in range(4)]

        def emit_va(srange):
            for mch in range(4):
                if srange[0] == 0:
                    nc.vector.memset(va_sb[mch][:, :, :, DH:DH + 1], 1.0)
                for s in srange:
                    pg = pp256.tile([P, L], f32, tag="p256")
                    for dc in range(2):
                        lhsT = kt_v[dc][:, s * 512 + mch * P:
                                        s * 512 + (mch + 1) * P]
                        mm(pg, lhsT, wv_sb[:, dc, :],
                           start=(dc == 0), stop=(dc == 1))
                    dst = va_sb[mch].rearrange("p h s j -> p s h j")[
                        :, s, :, 0:DH]
                    src = pg.rearrange("p (h j) -> p h j", j=DH)
                    if (mch + s) % 2 == 0:
                        nc.scalar.copy(dst, src)
                    else:
                        nc.vector.tensor_copy(dst, src)

        emit_va((0, 1))
        proj(qt_v, wq_sb, bq_c, qpt_sb, qpt8_sb, False, (0, 1))
        proj(kt_v, wk_sb, bk_c, None, kpt8_sb, True, (2, 3))
        emit_va((2, 3))
        proj(qt_v, wq_sb, bq_c, qpt_sb, qpt8_sb, False, (2, 3))



        # ---- QN = transpose(QpT) into o_sb early (plain copy / +bv) ----
        o_sb = [opool.tile([P, S, L], f32, tag="o", name=f"ot{i}") for i in range(4)]

        def emit_qn(nch):
            for half in range(2):
                ptb = ppb.tile([P, 512], b16, tag="p512b")
                for k in range(2):
                    s = half * 2 + k
                    for lc in range(2):
                        mm(ptb[:, (k * 2 + lc) * P:(k * 2 + lc + 1) * P],
                           qpt_sb[lc][:, s * 512 + nch * P: s * 512 + (nch + 1) * P],
                           ident_r, is_transpose=True, start=True, stop=True)
                o_flat = o_sb[nch][:, half * 2:(half + 1) * 2, :].rearrange(
                    "p s l -> p (s l)")
                if bias_zero:
                    if (nch + half) % 2 == 0:
                        nc.vector.tensor_copy(o_flat, ptb)
                    else:
                        nc.scalar.copy(o_flat, ptb)
                else:
                    nc.vector.tensor_add(
                        o_flat, ptb, bv_bc.rearrange("p s l -> p (s l)"))

        # ---- attention: E^T + exp per head; AVn fused with residual add ----
        ybf_sb = [ybfp.tile([P, S, L], b16, tag="ybf", name=f"ybf{i}")
                  for i in range(4)]
        at_tiles = {}

        def emit_e_pair(h0):
            # heads h0, h0+1 share an l-chunk; their K=64 matmuls sit on
            # disjoint SBUF partition halves -> row-tiled PE halves (T0/T8)
            # run them concurrently when interleaved
            lc = h0 // 2
            at_tiles[h0] = []
            at_tiles[h0 + 1] = []
            DR = mybir.MatmulPerfMode.DoubleRow
            for mc in range(4):
                pes = [ppw.tile([P, 512], f32, tag="pw", name="pe")
                       for _ in range(2)]
                for sp in range(2):
                    for hh in range(2):
                        r0 = hh * DH
                        lhsT = kpt8_sb[lc][r0:r0 + DH, sp, :,
                                           mc * P:(mc + 1) * P]
                        rhs = qpt8_sb[lc][r0:r0 + DH, sp, :, :]
                        mm(pes[hh], lhsT, rhs, perf_mode=DR,
                           start=(sp == 0), stop=(sp == 1))
                for hh in range(2):
                    if use_mask:
                        nc.vector.tensor_add(pes[hh], pes[hh], mb_sb[mc])
                    at = at_pool.tile([P, N], b16, tag="at", name="at")
                    nc.scalar.activation(at, pes[hh], AF.Exp, scale=1.0 / 16.0,
                                         bias=zero_col)
                    at_tiles[h0 + hh].append(at)

        def emit_avn(h, nch):
            pa = pp256.tile([P, S * (DH + 1)], f32, tag="p256", name="pa")
            for mc in range(4):
                lhsT = at_tiles[h][mc][:, nch * P:(nch + 1) * P]
                mm(pa, lhsT, va_sb[mc][:, h, :, :],
                   start=(mc == 0), stop=(mc == 3))
            rc = stat.tile([P, 1], f32, tag="rc", bufs=6, name="rc")
            nc.vector.reciprocal(rc, pa[:, DH:DH + 1])
            src = pa.rearrange("p (s j) -> p s j", j=DH + 1)[:, :, 0:DH]
            dst = o_sb[nch][:, :, h * DH:(h + 1) * DH]
            nc.vector.scalar_tensor_tensor(out=dst, in0=src, scalar=rc, in1=dst,
                                           op0=OP.mult, op1=OP.add)

        # LN0 stat tiles
        mv0 = stat.tile([P, 16, 2], f32)
        rstd0 = stat.tile([P, 16], f32)
        nmr0 = stat.tile([P, 16], f32)

        def emit_ln0(nch):
            i0 = nch * 4
            for s in range(S):
                st6 = stat.tile([P, 6], f32, tag="st6", bufs=4)
                nc.vector.bn_stats(out=st6, in_=o_sb[nch][:, s, :])
                nc.vector.bn_aggr(out=mv0[:, i0 + s, :], in_=st6)
            tmp0 = stat.tile([P, 4], f32, tag="tmp0", bufs=4)
            nc.scalar.activation(tmp0, mv0[:, i0:i0 + 4, 1], AF.Sqrt,
                                 bias=eps_col)
            nc.vector.reciprocal(rstd0[:, i0:i0 + 4], tmp0)
            nc.vector.scalar_tensor_tensor(
                out=nmr0[:, i0:i0 + 4], in0=mv0[:, i0:i0 + 4, 0], scalar=-1.0,
                in1=rstd0[:, i0:i0 + 4], op0=OP.mult, op1=OP.mult)
            for s in range(S):
                if nch < 2:
                    nc.scalar.activation(
                        ybf_sb[nch][:, s, :], o_sb[nch][:, s, :], AF.Identity,
                        scale=rstd0[:, i0 + s:i0 + s + 1],
                        bias=nmr0[:, i0 + s:i0 + s + 1])
                else:
                    nc.vector.tensor_scalar(
                        out=ybf_sb[nch][:, s, :], in0=o_sb[nch][:, s, :],
                        scalar1=rstd0[:, i0 + s:i0 + s + 1],
                        scalar2=nmr0[:, i0 + s:i0 + s + 1],
                        op0=OP.mult, op1=OP.add)

        # o1t layout: [l-chunk, (s n)]
        o1t_sb = [fmaj.tile([P, T], b16, tag="io", name=f"o1T{i}") for i in range(2)]

        def emit_o1t(nch):
            for lc in range(2):
                if lc == 0:
                    pt = ppb.tile([P, 512], b16, tag="p512b")
                else:
                    pt = ppw.tile([P, 512], b16, tag="pw")
                for s in range(S):
                    mm(pt[:, s * P:(s + 1) * P],
                       ybf_sb[nch][:, s, lc * P:(lc + 1) * P],
                       ident_r, is_transpose=True, start=True, stop=True)
                dst = o1t_sb[lc].rearrange("p (s n) -> p s n", s=S)[
                    :, :, nch * P:(nch + 1) * P]
                src = pt.rearrange("p (s n) -> p s n", s=S)
                if lc == 0:
                    nc.scalar.activation(dst, src, AF.Identity,
                                         bias=b0_c[:, lc:lc + 1],
                                         scale=g0_c[:, lc:lc + 1])
                else:
                    nc.vector.tensor_scalar(out=dst, in0=src,
                                            scalar1=g0_c[:, lc:lc + 1],
                                            scalar2=b0_c[:, lc:lc + 1],
                                            op0=OP.mult, op1=OP.add)

        emit_e_pair(0)
        for nch in range(4):
            emit_qn(nch)
        for nch in range(4):
            emit_avn(0, nch)
        emit_e_pair(2)
        for nch in range(4):
            emit_avn(1, nch)
        for nch in range(4):
            emit_avn(2, nch)
        emit_avn(3, 0)
        emit_ln0(0)
        emit_avn(3, 1)
        emit_ln0(1)
        emit_avn(3, 2)
        emit_ln0(2)
        emit_avn(3, 3)
        emit_ln0(3)
        warm(8)
        emit_o1t(0)
        emit_o1t(1)
        emit_o1t(2)
        emit_o1t(3)

        # ---- fc by s-half; O2 transpose-back + LN1 + out per (nch, half) ----
        rt_sb = [fmaj.tile([P, T], b16, tag="io", name=f"rT{i}")
                 for i in range(2)]
        o1_sb = [outp.tile([P, S, L], b16, tag="o1f", name=f"o1f{i}")
                 for i in range(4)]
        mv1 = stat.tile([P, 16, 2], f32)
        rstd1 = stat.tile([P, 16], f32)
        nmr1 = stat.tile([P, 16], f32)

        def emit_fc(tch):
            pfs = []
            for lpc in range(2):
                pf = ppw.tile([P, 512], f32, tag="pw")
                for lc in range(2):
                    mm(pf, wo_sb[:, lc, lpc * P:(lpc + 1) * P],
                       o1t_sb[lc][:, tch * 512:(tch + 1) * 512],
                       start=(lc == 0), stop=(lc == 1))
                pfs.append(pf)
            for lpc in range(2):
                sl = slice(tch * 512, (tch + 1) * 512)
                dst = rt_sb[lpc][:, sl]
                if lpc == 0:
                    nc.scalar.activation(dst, pfs[lpc], AF.Relu,
                                         bias=bo_c[:, lpc:lpc + 1])
                else:
                    nc.vector.tensor_scalar(out=dst, in0=pfs[lpc],
                                            scalar1=bo_c[:, lpc:lpc + 1],
                                            scalar2=0.0, op0=OP.add,
                                            op1=OP.max)

        def emit_o2(nch, half):
            i0 = nch * 4
            pt = ppw.tile([P, 512], f32, tag="pw")
            for k in range(2):
                s = half * 2 + k
                for lc in range(2):
                    blk = slice(s * 512 + nch * P, s * 512 + (nch + 1) * P)
                    out_blk = pt[:, (k * 2 + lc) * P:(k * 2 + lc + 1) * P]
                    mm(out_blk, o1t_sb[lc][:, blk], ident_r,
                       start=True, stop=False)
                    mm(out_blk, rt_sb[lc][:, blk], ident_r,
                       start=False, stop=True)
            for k in range(2):
                s = half * 2 + k
                st6 = stat.tile([P, 6], f32, tag="st6", bufs=4)
                nc.vector.bn_stats(out=st6, in_=pt[:, k * L:(k + 1) * L])
                nc.vector.bn_aggr(out=mv1[:, i0 + s, :], in_=st6)
            j0 = i0 + half * 2
            tmp1 = stat.tile([P, 2], f32, tag="tmp1", bufs=4)
            nc.scalar.activation(tmp1, mv1[:, j0:j0 + 2, 1], AF.Sqrt,
                                 bias=eps_col)
            nc.vector.reciprocal(rstd1[:, j0:j0 + 2], tmp1)
            nc.vector.scalar_tensor_tensor(
                out=nmr1[:, j0:j0 + 2], in0=mv1[:, j0:j0 + 2, 0], scalar=-1.0,
                in1=rstd1[:, j0:j0 + 2], op0=OP.mult, op1=OP.mult)
            for k in range(2):
                s = half * 2 + k
                nc.scalar.activation(
                    o1_sb[nch][:, s, :], pt[:, k * L:(k + 1) * L],
                    AF.Identity, scale=rstd1[:, i0 + s:i0 + s + 1],
                    bias=nmr1[:, i0 + s:i0 + s + 1])
                if not aff1_trivial:
                    if s % 2 == 0:
                        nc.vector.tensor_mul(o1_sb[nch][:, s, :],
                                             o1_sb[nch][:, s, :], g1_bc)
                        nc.gpsimd.tensor_add(o1_sb[nch][:, s, :],
                                             o1_sb[nch][:, s, :], b1_bc)
                    else:
                        nc.gpsimd.tensor_mul(o1_sb[nch][:, s, :],
                                             o1_sb[nch][:, s, :], g1_bc)
                        nc.vector.tensor_add(o1_sb[nch][:, s, :],
                                             o1_sb[nch][:, s, :], b1_bc)
            if half == 1:
                nc.sync.dma_start(
                    out=out_d[nch * P:(nch + 1) * P, :],
                    in_=o1_sb[nch].rearrange("p s l -> p (s l)"))

        emit_fc(0)
        emit_fc(1)
        emit_o2(0, 0)
        emit_fc(2)
        emit_o2(1, 0)
        emit_fc(3)
        emit_o2(2, 0)
        emit_o2(0, 1)
        emit_o2(3, 0)
        emit_o2(1, 1)
        emit_o2(2, 1)
        emit_o2(3, 1)

    nc.compile()
    return nc


def kernel(**inputs):
    global _CACHE
    Q = np.asarray(inputs["Q"], dtype=np.float32)
    K = np.asarray(inputs["K"], dtype=np.float32)
    mask = np.asarray(inputs["mask"])
    Wq = np.asarray(inputs["Wq"], dtype=np.float32)
    bq = np.asarray(inputs["bq"], dtype=np.float32)
    Wk = np.asarray(inputs["Wk"], dtype=np.float32)
    bk = np.asarray(inputs["bk"], dtype=np.float32)
    Wv = np.asarray(inputs["Wv"], dtype=np.float32)
    bv = np.asarray(inputs["bv"], dtype=np.float32)
    Wo = np.asarray(inputs["Wo"], dtype=np.float32)
    bo = np.asarray(inputs["bo"], dtype=np.float32)
    g0 = np.asarray(inputs["g0"], dtype=np.float32)
    b0 = np.asarray(inputs["b0"], dtype=np.float32)
    g1 = np.asarray(inputs["g1"], dtype=np.float32)
    b1 = np.asarray(inputs["b1"], dtype=np.float32)

    use_mask = not bool((mask != 0).all())
    bias_zero = bool((bv == 0.0).all())
    aff1_trivial = bool((g1 == 1.0).all() and (b1 == 0.0).all())

    from concourse.bass_utils import run_bass_kernel_spmd

    key = ("nc", use_mask, bias_zero, aff1_trivial)
    if key not in _CACHE:
        _CACHE[key] = _build(use_mask, bias_zero, aff1_trivial)
    nc = _CACHE[key]

    import ml_dtypes
    bf16 = ml_dtypes.bfloat16
    cst = np.zeros((P, 10), np.float32)
    cst[:, 0:2] = bq.reshape(2, P).T
    cst[:, 2:4] = bk.reshape(2, P).T
    cst[:, 4:6] = bo.reshape(2, P).T
    cst[:, 6:8] = g0.reshape(2, P).T
    cst[:, 8:10] = b0.reshape(2, P).T
    common = {
        "wall": np.ascontiguousarray(
            np.stack([Wq.T, Wk.T, Wv.T, Wo.T])  # [w, d, l]
            .reshape(4, 2, P, L).transpose(2, 1, 0, 3)  # [p, c, w, l]
            .reshape(P, 2 * 4 * L)).astype(bf16),
        "cst": cst,
    }
    if not bias_zero:
        common["bvr"] = bv.astype(bf16)
    if not aff1_trivial:
        common["g1v"] = g1
        common["b1v"] = b1
    if use_mask:
        common["mbt"] = np.ascontiguousarray(
            np.where(mask == 0, np.float32(-1e30), np.float32(0.0)).T)

    in_maps = []
    for b in range(NCORES):
        m = dict(common)
        # [D, (s n)]: column index = s*N + n  (s-major token order)
        m["qt"] = np.ascontiguousarray(
            Q[b].transpose(2, 1, 0).reshape(D, T)).astype(bf16)
        m["kt"] = np.ascontiguousarray(
            K[b].transpose(2, 1, 0).reshape(D, T)).astype(bf16)
        in_maps.append(m)

    trace = os.environ.get("KERNEL_TRACE", "0") == "1"
    res = run_bass_kernel_spmd(nc, in_maps, core_ids=list(range(NCORES)),
                               trace=trace)
    globals()["LAST_RESULTS"] = res
    out = np.stack([
        res.results[b]["o"].astype(np.float32).reshape(N, S, L)
        for b in range(NCORES)])
    return out
